# revision 1
# baseline (speedup 1.0000x reference)
"""Trainium2 Bass kernel for nn_K_attention_12086037971047.

out[b] = x[b] + Km[b] @ x[b],  Km = exp(-r_sigma * d2(x_b)) with zero diagonal.

Key identity: Km = diag(a) . E . diag(a) with
  a_i = exp(-sigma*||x_i||^2),  E = exp(2*sigma * x x^T)  (E symmetric).
Masked-diagonal output:
  out = coef (.) x + a (.) (E @ (a (.) x)),   coef_i = 1 - a_i^2 * exp(2*sigma*||x_i||^2)
(the coef term subtracts the j==i contribution of the unmasked sum).

Sharding: data-parallel over B: 16 batches -> 8 cores x 2 batches.

Per batch on each core (T=2048, C=64, P=128):
  phase 1: G row-block [128 x 2048] = x_blk x^T via f32r matmuls (K=C=64),
           E row = exp(2 sigma G) on the ACT engine (PSUM -> SBUF)
  phase 2: zT [64 x 2048] += y_blk^T . E_row (f32r, K=128), accumulated in PSUM
           over the 16 row blocks (zT = (E @ y)^T by symmetry of E)
  epilogue: transpose zT back to row layout on the PE, combine with coef/a,
           DMA out contiguously.
"""

import numpy as np

import concourse.bass as bass
import concourse.mybir as mybir
import concourse.tile as tile
from concourse import bacc
from concourse.bass_utils import run_bass_kernel_spmd
from concourse.masks import make_identity

B, T, C = 16, 2048, 64
N_CORES = 8
B_LOC = B // N_CORES  # batches per core
P = 128
NB = T // P  # 16 row blocks
FC = 512  # psum chunk (one 2KB fp32 bank)
NCH = T // FC  # 4 chunks

F32 = mybir.dt.float32
F32R = mybir.dt.float32r
AF = mybir.ActivationFunctionType
OP = mybir.AluOpType


def _emit(tc: tile.TileContext, x, rs, out, reps: int = 1):
    nc = tc.nc
    import contextlib

    with contextlib.ExitStack() as ctx:
        singles = ctx.enter_context(tc.tile_pool(name="singles", bufs=1))
        sb = ctx.enter_context(tc.tile_pool(name="sb", bufs=2))
        ps = ctx.enter_context(tc.tile_pool(name="ps", bufs=1, space="PSUM"))

        # --- constants ---
        sig = singles.tile([P, 1], F32)
        nc.sync.dma_start(sig, rs[:].to_broadcast([P, 1]))
        neg_sig = singles.tile([P, 1], F32)
        nc.scalar.mul(neg_sig, sig, -1.0)
        two_sig = singles.tile([P, 1], F32)
        nc.scalar.mul(two_sig, sig, 2.0)
        ident = singles.tile([P, P], F32)
        make_identity(nc, ident)

        for b in [bb for _ in range(reps) for bb in range(B_LOC)]:
            # --- load x in row layout: partition p holds rows o*128+p ---
            x_rows = sb.tile([P, NB, C], F32, tag="x_rows")
            nc.sync.dma_start(x_rows, x[b].rearrange("(o p) c -> p o c", p=P))

            # --- per-row stats: sq, a=exp(-s*sq), coef = 1 - a^2*exp(2s*sq) ---
            xsq = sb.tile([P, NB, C], F32, tag="xsq")
            nc.vector.tensor_mul(xsq, x_rows, x_rows)
            sq = sb.tile([P, NB], F32, tag="sq")
            nc.vector.tensor_reduce(sq, xsq, axis=mybir.AxisListType.X, op=OP.add)
            a_t = sb.tile([P, NB], F32, tag="a_t")
            nc.scalar.activation(a_t, sq, AF.Exp, scale=neg_sig)
            e_diag = sb.tile([P, NB], F32, tag="e_diag")
            nc.scalar.activation(e_diag, sq, AF.Exp, scale=two_sig)
            coef = sb.tile([P, NB], F32, tag="coef")
            nc.vector.tensor_mul(coef, a_t, a_t)
            nc.vector.tensor_mul(coef, coef, e_diag)
            # coef = 1 - coef
            nc.vector.tensor_scalar(coef, coef, -1.0, 1.0, OP.mult, OP.add)

            # --- y = a (.) x (row-scaled) ---
            y_t = sb.tile([P, NB, C], F32R, tag="y_t")
            nc.vector.tensor_tensor(
                y_t, x_rows, a_t[:, :, None].to_broadcast([P, NB, C]), OP.mult
            )

            # --- xT [C, T] via PE transposes ---
            xT = sb.tile([C, T], F32R, tag="xT")
            for o in range(NB):
                tp = ps.tile([C, P], F32, tag="tp", bufs=2)
                nc.tensor.transpose(tp, x_rows[:, o, :], ident)
                nc.vector.tensor_copy(xT[:, o * P : (o + 1) * P], tp)

            # --- main loop: E row blocks + zT accumulation ---
            zT = ps.tile([C, T], F32, tag="zT", bufs=1)
            for o in range(NB):
                e_sb = sb.tile([P, T], F32R, tag="e_sb")
                for ch in range(NCH):
                    g_ps = ps.tile([P, FC], F32, tag="g", bufs=2)
                    nc.tensor.matmul(
                        g_ps,
                        lhsT=xT[:, o * P : (o + 1) * P],
                        rhs=xT[:, ch * FC : (ch + 1) * FC],
                        start=True,
                        stop=True,
                    )
                    nc.scalar.activation(
                        e_sb[:, ch * FC : (ch + 1) * FC], g_ps, AF.Exp, scale=two_sig
                    )
                for ch in range(NCH):
                    nc.tensor.matmul(
                        zT[:, ch * FC : (ch + 1) * FC],
                        lhsT=y_t[:, o, :],
                        rhs=e_sb[:, ch * FC : (ch + 1) * FC],
                        start=(o == 0),
                        stop=(o == NB - 1),
                        skip_group_check=True,
                    )

            # --- epilogue: zT -> rows, combine, store ---
            zT_sb = sb.tile([C, T], F32, tag="zT_sb")
            nc.vector.tensor_copy(zT_sb, zT)
            z_rows = sb.tile([P, NB, C], F32, tag="z_rows")
            for o in range(NB):
                tp2 = ps.tile([P, C], F32, tag="tp", bufs=2)
                nc.tensor.transpose(tp2, zT_sb[:, o * P : (o + 1) * P], ident[:C, :C])
                nc.vector.tensor_copy(z_rows[:, o, :], tp2)

            out_sb = sb.tile([P, NB, C], F32, tag="out_sb")
            nc.vector.tensor_tensor(
                out_sb, z_rows, a_t[:, :, None].to_broadcast([P, NB, C]), OP.mult
            )
            xc = sb.tile([P, NB, C], F32, tag="xc")
            nc.vector.tensor_tensor(
                xc, x_rows, coef[:, :, None].to_broadcast([P, NB, C]), OP.mult
            )
            nc.vector.tensor_add(out_sb, out_sb, xc)
            nc.sync.dma_start(out[b].rearrange("(o p) c -> p o c", p=P), out_sb)


def build(reps: int = 1):
    nc = bacc.Bacc("TRN2", target_bir_lowering=False)
    x = nc.dram_tensor("x", [B_LOC, T, C], F32, kind="ExternalInput")
    rs = nc.dram_tensor("r_sigma", [1], F32, kind="ExternalInput")
    out = nc.dram_tensor("out", [B_LOC, T, C], F32, kind="ExternalOutput")
    with tile.TileContext(nc) as tc:
        _emit(tc, x, rs, out, reps=reps)
    nc.compile()
    return nc


_NC = None


def _get_nc():
    global _NC
    if _NC is None:
        _NC = build()
    return _NC


def kernel(x: np.ndarray, r_sigma: np.ndarray) -> np.ndarray:
    x = np.ascontiguousarray(x, dtype=np.float32)
    r_sigma = np.ascontiguousarray(r_sigma, dtype=np.float32)
    nc = _get_nc()
    in_maps = [
        {"x": x[i * B_LOC : (i + 1) * B_LOC], "r_sigma": r_sigma}
        for i in range(N_CORES)
    ]
    res = run_bass_kernel_spmd(nc, in_maps, core_ids=list(range(N_CORES)))
    return np.concatenate([r["out"] for r in res.results], axis=0)

